# revision 3
# baseline (speedup 1.0000x reference)
"""Multi-head distance (attention) layer on 8 TRN2 NeuronCores.

Sharding: data-parallel over batch. B=8 -> one batch element per core.
Each core computes a full multi-head self-attention for its [L=1024, D=256]
slice with H=8 heads of dim 64. No collectives needed.

Per-core algorithm (all layouts chosen so softmax needs no transposes):
  xT   = transpose(x)            (PE transposes, fp32)
  qkT  = xT + peT                (pos-enc, host-precomputed constant, DVE)
  qT   = Wq.T @ x_pe             via matmul(lhsT=Wq, rhs=qkT)    [f32r]
  kT   = Wk.T @ x_pe             "
  v    = x @ Wv                  via matmul(lhsT=xT, rhs=Wv)     [f32r]
  per head h:
    sT[m,l] = sum_d kT[d,m] qT[d,l]      matmul(lhsT=kT_h, rhs=qT_h) [f32r]
    eT      = exp(0.125 * sT)            ScalarE, PSUM->SBUF, bf16 out
    O[l,d]+Z = eT.T @ [v_h | 1]          matmul(lhsT=eT, rhs=v_aug)  [bf16]
    out_h   = O * (1/Z)                  DVE reciprocal + tensor_scalar
Bias handling: bq added to qT during PSUM drain (per-partition scalar);
bk is softmax-invariant (adds a per-row constant to scores) so dropped;
bv shifts the output by exactly repeat(bv, 64) (softmax rows sum to 1),
added on the host.
"""

import numpy as np

import concourse.bass as bass
import concourse.mybir as mybir
import concourse.tile as tile
from concourse import bacc
from concourse.bass_utils import run_bass_kernel_spmd
from concourse.masks import make_identity

B, L, D = 8, 1024, 256
H, HD = 8, 64
J = H * HD  # 512
TEMPERATURE = 10000.0

f32 = mybir.dt.float32
f32r = mybir.dt.float32r
bf16 = mybir.dt.bfloat16

_CACHE = {}
LAST_RESULT = None  # BassKernelResults of the most recent run (for profiling)
TRACE = False


def _emit(tc, aps):
    nc = tc.nc
    Exp = mybir.ActivationFunctionType.Exp
    x, wq, wk, wv, bqc, pet, out = (
        aps["x"], aps["wq"], aps["wk"], aps["wv"], aps["bqc"], aps["pet"], aps["out"],
    )

    # DRAM views: partition-major tilings
    xr = x.rearrange("(n p) c -> p n c", p=128)          # [128, 8, 256]
    petr = pet.rearrange("(t p) l -> t p l", p=128)      # [2, 128, 1024]
    wqr = wq.rearrange("(t p) j -> t p j", p=128)        # [2, 128, 512]
    wkr = wk.rearrange("(t p) j -> t p j", p=128)
    wvr = wv.rearrange("(t p) j -> t p j", p=128)
    outr = out.rearrange("(n p) j -> p n j", p=128)      # [128, 8, 512]

    import contextlib
    ctx = contextlib.ExitStack()
    persist = ctx.enter_context(tc.tile_pool(name="persist", bufs=1))
    epool = ctx.enter_context(tc.tile_pool(name="epool", bufs=20))
    rpool = ctx.enter_context(tc.tile_pool(name="rpool", bufs=4))
    mm_ps = ctx.enter_context(tc.tile_pool(name="mmps", bufs=2, space="PSUM"))
    s_ps = ctx.enter_context(tc.tile_pool(name="sps", bufs=2, space="PSUM"))
    o_ps = ctx.enter_context(tc.tile_pool(name="ops", bufs=2, space="PSUM"))

    # --- constants / inputs to SBUF ---
    ident = persist.tile([128, 128], f32, name="ident")
    make_identity(nc, ident)
    bq_sb = persist.tile([128, 4], f32, name="bq_sb")
    nc.sync.dma_start(out=bq_sb[:], in_=bqc[:, :])

    x_sb = persist.tile([128, 8, 256], f32, name="x_sb")
    for n in range(8):
        for c2 in range(2):
            sl = slice(c2 * 128, (c2 + 1) * 128)
            nc.sync.dma_start(out=x_sb[:, n, sl], in_=xr[:, n, sl])

    pe_sb = [persist.tile([128, 1024], f32, name=f"pe_sb{t}") for t in range(2)]
    for t in range(2):
        for q in range(4):
            sl = slice(q * 256, (q + 1) * 256)
            nc.sync.dma_start(out=pe_sb[t][:, sl], in_=petr[t, :, sl])

    w_sb = {}
    for wname, wr in (("wq", wqr), ("wk", wkr), ("wv", wvr)):
        w_sb[wname] = [
            persist.tile([128, 512], f32r, name=f"{wname}_sb{t}") for t in range(2)
        ]
        for t in range(2):
            for hhalf in range(2):
                sl = slice(hhalf * 256, (hhalf + 1) * 256)
                nc.sync.dma_start(
                    out=w_sb[wname][t][:, sl], in_=wr[t, :, sl].bitcast(f32r)
                )

    # --- transpose x; build qkT = xT + peT ---
    xT = [persist.tile([128, 1024], f32r, name=f"xT{t}") for t in range(2)]
    qkT = [persist.tile([128, 1024], f32r, name=f"qkT{t}") for t in range(2)]
    for n in range(8):
        for c2 in range(2):
            tp = mm_ps.tile([128, 512], f32, tag="mm", name="tp")
            nc.tensor.transpose(
                tp[:, 0:128], x_sb[:, n, c2 * 128:(c2 + 1) * 128], ident[:]
            )
            dsl = slice(n * 128, (n + 1) * 128)
            nc.vector.tensor_copy(xT[c2][:, dsl], tp[:, 0:128])
            nc.vector.tensor_add(qkT[c2][:, dsl], tp[:, 0:128], pe_sb[c2][:, dsl])

    # --- QKV projections ---
    qT = [persist.tile([128, 1024], f32r, name=f"qT{j}") for j in range(4)]
    kT = [persist.tile([128, 1024], f32r, name=f"kT{j}") for j in range(4)]
    v_sb = [persist.tile([128, 8, 65], bf16, name=f"v_sb{m}") for m in range(8)]

    def qk_proj(j, which):
        dst, wname = (qT, "wq") if which == "q" else (kT, "wk")
        for l2 in range(2):
            pq = mm_ps.tile([128, 512], f32, tag="mm", name="pq")
            for c2 in range(2):
                nc.tensor.matmul(
                    pq[:],
                    lhsT=w_sb[wname][c2][:, j * 128:(j + 1) * 128],
                    rhs=qkT[c2][:, l2 * 512:(l2 + 1) * 512],
                    start=(c2 == 0),
                    stop=(c2 == 1),
                )
            dsl = slice(l2 * 512, (l2 + 1) * 512)
            if which == "q":
                nc.vector.tensor_scalar_add(dst[j][:, dsl], pq[:], bq_sb[:, j:j + 1])
            else:
                nc.vector.tensor_copy(dst[j][:, dsl], pq[:])

    def v_proj(m):
        pv = mm_ps.tile([128, 512], f32, tag="mm", name="pv")
        for c2 in range(2):
            nc.tensor.matmul(
                pv[:],
                lhsT=xT[c2][:, m * 128:(m + 1) * 128],
                rhs=w_sb["wv"][c2][:],
                start=(c2 == 0),
                stop=(c2 == 1),
            )
        nc.vector.tensor_copy(
            v_sb[m][:, :, 0:64], pv.rearrange("p (h d) -> p h d", h=8)
        )
        nc.vector.memset(v_sb[m][:, :, 64:65], 1.0)

    # head 0/1 operands first, then V, then the rest
    qk_proj(0, "q")
    qk_proj(0, "k")
    for m in range(8):
        v_proj(m)
    for j in range(1, 4):
        qk_proj(j, "q")
        qk_proj(j, "k")

    # --- attention per head ---
    out_sb = persist.tile([128, 8, 512], f32, name="out_sb")
    for h in range(H):
        j2, po = h // 2, 64 * (h % 2)
        es = []
        for mc in range(8):
            ps = s_ps.tile([128, 1024], f32, tag="s", name="ps")
            for l2 in range(2):
                nc.tensor.matmul(
                    ps[:, l2 * 512:(l2 + 1) * 512],
                    lhsT=kT[j2][po:po + 64, mc * 128:(mc + 1) * 128],
                    rhs=qT[j2][po:po + 64, l2 * 512:(l2 + 1) * 512],
                    start=True,
                    stop=True,
                )
            e = epool.tile([128, 1024], bf16, tag="e", name="e")
            nc.scalar.activation(e[:], ps[:], Exp, scale=float(HD) ** -0.5)
            es.append(e)
        for lc in range(8):
            pO = o_ps.tile([128, 65], f32, tag="o", name="pO")
            for mc in range(8):
                nc.tensor.matmul(
                    pO[:],
                    lhsT=es[mc][:, lc * 128:(lc + 1) * 128],
                    rhs=v_sb[mc][:, h, :],
                    start=(mc == 0),
                    stop=(mc == 7),
                )
            rc = rpool.tile([128, 1], f32, tag="rc", name="rc")
            nc.vector.reciprocal(rc[:], pO[:, 64:65])
            nc.vector.tensor_scalar_mul(
                out_sb[:, lc, h * 64:(h + 1) * 64], pO[:, 0:64], rc[:]
            )
        hsl = slice(h * 64, (h + 1) * 64)
        for g in range(4):
            nc.sync.dma_start(
                out=outr[:, 2 * g:2 * g + 2, hsl],
                in_=out_sb[:, 2 * g:2 * g + 2, hsl],
            )
    ctx.close()


def _build():
    if "nc" in _CACHE:
        return _CACHE["nc"]
    nc = bacc.Bacc("TRN2", target_bir_lowering=False, debug=False, num_devices=8)
    aps = {
        "x": nc.dram_tensor("x", [L, D], f32, kind="ExternalInput").ap(),
        "wq": nc.dram_tensor("wq", [D, J], f32, kind="ExternalInput").ap(),
        "wk": nc.dram_tensor("wk", [D, J], f32, kind="ExternalInput").ap(),
        "wv": nc.dram_tensor("wv", [D, J], f32, kind="ExternalInput").ap(),
        "bqc": nc.dram_tensor("bqc", [128, 4], f32, kind="ExternalInput").ap(),
        "pet": nc.dram_tensor("pet", [D, L], f32, kind="ExternalInput").ap(),
        "out": nc.dram_tensor("out", [L, J], f32, kind="ExternalOutput").ap(),
    }
    with tile.TileContext(nc) as tc:
        _emit(tc, aps)
    nc.compile()
    _CACHE["nc"] = nc
    return nc


def _pe_T():
    embed = np.arange(L, dtype=np.float32)
    dim_t = np.arange(D, dtype=np.float32)
    dim_t = (np.float32(TEMPERATURE) ** (2.0 * np.floor(dim_t / 2.0) / np.float32(D))).astype(np.float32)
    pos = embed[:, None] / dim_t  # [L, D]
    pe = np.stack([np.sin(pos[:, 0::2]), np.cos(pos[:, 1::2])], axis=2).reshape(L, D)
    return np.ascontiguousarray(pe.T.astype(np.float32))  # [D, L]


def kernel(**inputs):
    global LAST_RESULT
    x = np.ascontiguousarray(np.asarray(inputs["x"], dtype=np.float32))
    wq = np.ascontiguousarray(np.asarray(inputs["Wq"], dtype=np.float32))
    wk = np.ascontiguousarray(np.asarray(inputs["Wk"], dtype=np.float32))
    wv = np.ascontiguousarray(np.asarray(inputs["Wv"], dtype=np.float32))
    bq = np.asarray(inputs["bq"], dtype=np.float32)
    bv = np.asarray(inputs["bv"], dtype=np.float32)

    nc = _build()
    bqc = np.ascontiguousarray(np.repeat(bq, HD).reshape(4, 128).T)  # [128, 4]
    pet = _pe_T()
    base = {"wq": wq, "wk": wk, "wv": wv, "bqc": bqc, "pet": pet}
    in_maps = [{**base, "x": np.ascontiguousarray(x[b])} for b in range(B)]
    res = run_bass_kernel_spmd(
        nc, in_maps, core_ids=list(range(B)), trace=TRACE
    )
    LAST_RESULT = res
    out = np.stack([res.results[b]["out"] for b in range(B)]).astype(np.float32)
    out += np.repeat(bv, HD)[None, None, :]
    return out


# revision 4
# speedup vs baseline: 1.3594x; 1.3594x over previous
"""Multi-head distance (attention) layer on 8 TRN2 NeuronCores.

Sharding: data-parallel over batch. B=8 -> one batch element per core.
Each core computes a full multi-head self-attention for its [L=1024, D=256]
slice with H=8 heads of dim 64. No collectives needed.

Per-core algorithm (all layouts chosen so softmax needs no transposes and
all matmul operands are bf16 so the PE streams at 1 row/cycle at any clock):
  xT   = dma-transpose(x)        (hardware xbar transpose, bf16)
  qkT  = xT + peT                (pos-enc, host-precomputed constant, DVE)
  qT   = Wq.T @ x_pe             via matmul(lhsT=Wq, rhs=qkT)
  kT   = Wk.T @ x_pe             "
  v    = x @ Wv                  via matmul(lhsT=xT, rhs=Wv)
  per head h:
    sT[m,l] = sum_d kT[d,m] qT[d,l]      matmul(lhsT=kT_h, rhs=qT_h)
    eT      = exp(0.125 * sT)            ScalarE, PSUM->SBUF, bf16 out
    O[l,d]+Z = eT.T @ [v_h | 1]          matmul(lhsT=eT, rhs=v_aug)
    out_h   = O * (1/Z)                  DVE reciprocal + tensor_scalar
Bias handling: bq added to qT during PSUM drain (per-partition scalar, in
fp32 before the bf16 rounding); bk only shifts each score row by a constant
(softmax-invariant) so it is dropped; bv shifts the output by exactly
repeat(bv, 64) because softmax rows sum to 1, added on the host.
"""

import numpy as np
import ml_dtypes

import concourse.bass as bass
import concourse.mybir as mybir
import concourse.tile as tile
from concourse import bacc
from concourse.bass_utils import run_bass_kernel_spmd

B, L, D = 8, 1024, 256
H, HD = 8, 64
J = H * HD  # 512
TEMPERATURE = 10000.0

f32 = mybir.dt.float32
bf16 = mybir.dt.bfloat16

_CACHE = {}
LAST_RESULT = None  # BassKernelResults of the most recent run (for profiling)
TRACE = False


def _emit(tc, aps):
    nc = tc.nc
    Exp = mybir.ActivationFunctionType.Exp
    x, wq, wk, wv, bqc, pet, out = (
        aps["x"], aps["wq"], aps["wk"], aps["wv"], aps["bqc"], aps["pet"], aps["out"],
    )

    # DRAM views: partition-major tilings
    petr = pet.rearrange("(t p) l -> t p l", p=128)      # [2, 128, 1024]
    wqr = wq.rearrange("(t p) j -> t p j", p=128)        # [2, 128, 512]
    wkr = wk.rearrange("(t p) j -> t p j", p=128)
    wvr = wv.rearrange("(t p) j -> t p j", p=128)
    outr = out.rearrange("(n p) j -> p n j", p=128)      # [128, 8, 512]

    import contextlib
    ctx = contextlib.ExitStack()
    persist = ctx.enter_context(tc.tile_pool(name="persist", bufs=1))
    epool = ctx.enter_context(tc.tile_pool(name="epool", bufs=20))
    rpool = ctx.enter_context(tc.tile_pool(name="rpool", bufs=4))
    mm_ps = ctx.enter_context(tc.tile_pool(name="mmps", bufs=2, space="PSUM"))
    s_ps = ctx.enter_context(tc.tile_pool(name="sps", bufs=2, space="PSUM"))
    o_ps = ctx.enter_context(tc.tile_pool(name="ops", bufs=2, space="PSUM"))

    # --- inputs to SBUF ---
    # x transposed via the hardware xbar DMA transpose (bf16-only path).
    xT = [persist.tile([128, 1024], bf16, name=f"xT{t}") for t in range(2)]
    for t in range(2):
        nc.sync.dma_start_transpose(xT[t][:], x[:, t * 128:(t + 1) * 128])

    bq_sb = persist.tile([128, 4], f32, name="bq_sb")
    nc.sync.dma_start(out=bq_sb[:], in_=bqc[:, :])

    pe_sb = [persist.tile([128, 1024], bf16, name=f"pe_sb{t}") for t in range(2)]
    for t in range(2):
        for q in range(2):
            sl = slice(q * 512, (q + 1) * 512)
            nc.sync.dma_start(out=pe_sb[t][:, sl], in_=petr[t, :, sl])

    w_sb = {}
    for wname, wr in (("wq", wqr), ("wk", wkr), ("wv", wvr)):
        w_sb[wname] = [
            persist.tile([128, 512], bf16, name=f"{wname}_sb{t}") for t in range(2)
        ]
        for t in range(2):
            nc.sync.dma_start(out=w_sb[wname][t][:], in_=wr[t])

    # --- qkT = xT + peT ---
    qkT = [persist.tile([128, 1024], bf16, name=f"qkT{t}") for t in range(2)]
    for t in range(2):
        nc.vector.tensor_add(qkT[t][:], xT[t][:], pe_sb[t][:])

    # --- QKV projections ---
    qT = [persist.tile([128, 1024], bf16, name=f"qT{j}") for j in range(4)]
    kT = [persist.tile([128, 1024], bf16, name=f"kT{j}") for j in range(4)]
    v_sb = [persist.tile([128, 8, 65], bf16, name=f"v_sb{m}") for m in range(8)]

    def qk_proj(j, which):
        dst, wname = (qT, "wq") if which == "q" else (kT, "wk")
        for l2 in range(2):
            pq = mm_ps.tile([128, 512], f32, tag="mm", name="pq")
            for c2 in range(2):
                nc.tensor.matmul(
                    pq[:],
                    lhsT=w_sb[wname][c2][:, j * 128:(j + 1) * 128],
                    rhs=qkT[c2][:, l2 * 512:(l2 + 1) * 512],
                    start=(c2 == 0),
                    stop=(c2 == 1),
                )
            dsl = slice(l2 * 512, (l2 + 1) * 512)
            if which == "q":
                nc.vector.tensor_scalar_add(dst[j][:, dsl], pq[:], bq_sb[:, j:j + 1])
            else:
                nc.vector.tensor_copy(dst[j][:, dsl], pq[:])

    def v_proj(m):
        pv = mm_ps.tile([128, 512], f32, tag="mm", name="pv")
        for c2 in range(2):
            nc.tensor.matmul(
                pv[:],
                lhsT=xT[c2][:, m * 128:(m + 1) * 128],
                rhs=w_sb["wv"][c2][:],
                start=(c2 == 0),
                stop=(c2 == 1),
            )
        nc.vector.tensor_copy(
            v_sb[m][:, :, 0:64], pv.rearrange("p (h d) -> p h d", h=8)
        )
        nc.vector.memset(v_sb[m][:, :, 64:65], 1.0)

    # head 0/1 operands first, then V, then the rest
    qk_proj(0, "q")
    qk_proj(0, "k")
    for m in range(8):
        v_proj(m)
    for j in range(1, 4):
        qk_proj(j, "q")
        qk_proj(j, "k")

    # --- attention per head ---
    out_sb = persist.tile([128, 8, 512], f32, name="out_sb")
    for h in range(H):
        j2, po = h // 2, 64 * (h % 2)
        es = []
        for mc in range(8):
            ps = s_ps.tile([128, 1024], f32, tag="s", name="ps")
            for l2 in range(2):
                nc.tensor.matmul(
                    ps[:, l2 * 512:(l2 + 1) * 512],
                    lhsT=kT[j2][po:po + 64, mc * 128:(mc + 1) * 128],
                    rhs=qT[j2][po:po + 64, l2 * 512:(l2 + 1) * 512],
                    start=True,
                    stop=True,
                )
            e = epool.tile([128, 1024], bf16, tag="e", name="e")
            nc.scalar.activation(e[:], ps[:], Exp, scale=float(HD) ** -0.5)
            es.append(e)
        for lc in range(8):
            pO = o_ps.tile([128, 65], f32, tag="o", name="pO")
            for mc in range(8):
                nc.tensor.matmul(
                    pO[:],
                    lhsT=es[mc][:, lc * 128:(lc + 1) * 128],
                    rhs=v_sb[mc][:, h, :],
                    start=(mc == 0),
                    stop=(mc == 7),
                )
            rc = rpool.tile([128, 1], f32, tag="rc", name="rc")
            nc.vector.reciprocal(rc[:], pO[:, 64:65])
            nc.vector.tensor_scalar_mul(
                out_sb[:, lc, h * 64:(h + 1) * 64], pO[:, 0:64], rc[:]
            )
        hsl = slice(h * 64, (h + 1) * 64)
        for g in range(4):
            nc.sync.dma_start(
                out=outr[:, 2 * g:2 * g + 2, hsl],
                in_=out_sb[:, 2 * g:2 * g + 2, hsl],
            )
    ctx.close()


def _build():
    if "nc" in _CACHE:
        return _CACHE["nc"]
    nc = bacc.Bacc("TRN2", target_bir_lowering=False, debug=False, num_devices=8)
    aps = {
        "x": nc.dram_tensor("x", [L, D], bf16, kind="ExternalInput").ap(),
        "wq": nc.dram_tensor("wq", [D, J], bf16, kind="ExternalInput").ap(),
        "wk": nc.dram_tensor("wk", [D, J], bf16, kind="ExternalInput").ap(),
        "wv": nc.dram_tensor("wv", [D, J], bf16, kind="ExternalInput").ap(),
        "bqc": nc.dram_tensor("bqc", [128, 4], f32, kind="ExternalInput").ap(),
        "pet": nc.dram_tensor("pet", [D, L], bf16, kind="ExternalInput").ap(),
        "out": nc.dram_tensor("out", [L, J], f32, kind="ExternalOutput").ap(),
    }
    with tile.TileContext(nc) as tc:
        _emit(tc, aps)
    nc.compile()
    _CACHE["nc"] = nc
    return nc


def _pe_T():
    embed = np.arange(L, dtype=np.float32)
    dim_t = np.arange(D, dtype=np.float32)
    dim_t = (np.float32(TEMPERATURE) ** (2.0 * np.floor(dim_t / 2.0) / np.float32(D))).astype(np.float32)
    pos = embed[:, None] / dim_t  # [L, D]
    pe = np.stack([np.sin(pos[:, 0::2]), np.cos(pos[:, 1::2])], axis=2).reshape(L, D)
    return np.ascontiguousarray(pe.T.astype(np.float32))  # [D, L]


def kernel(**inputs):
    global LAST_RESULT
    bf = ml_dtypes.bfloat16
    x = np.asarray(inputs["x"], dtype=np.float32).astype(bf)
    wq = np.ascontiguousarray(np.asarray(inputs["Wq"], dtype=np.float32).astype(bf))
    wk = np.ascontiguousarray(np.asarray(inputs["Wk"], dtype=np.float32).astype(bf))
    wv = np.ascontiguousarray(np.asarray(inputs["Wv"], dtype=np.float32).astype(bf))
    bq = np.asarray(inputs["bq"], dtype=np.float32)
    bv = np.asarray(inputs["bv"], dtype=np.float32)

    nc = _build()
    bqc = np.ascontiguousarray(np.repeat(bq, HD).reshape(4, 128).T)  # [128, 4]
    pet = _pe_T().astype(bf)
    base = {"wq": wq, "wk": wk, "wv": wv, "bqc": bqc, "pet": pet}
    in_maps = [{**base, "x": np.ascontiguousarray(x[b])} for b in range(B)]
    res = run_bass_kernel_spmd(
        nc, in_maps, core_ids=list(range(B)), trace=TRACE
    )
    LAST_RESULT = res
    out = np.stack([res.results[b]["out"] for b in range(B)]).astype(np.float32)
    out += np.repeat(bv, HD)[None, None, :]
    return out


# revision 6
# speedup vs baseline: 1.4414x; 1.0603x over previous
"""Multi-head distance (attention) layer on 8 TRN2 NeuronCores.

Sharding: data-parallel over batch. B=8 -> one batch element per core.
Each core computes a full multi-head self-attention for its [L=1024, D=256]
slice with H=8 heads of dim 64. No collectives needed.

Per-core algorithm (all layouts chosen so softmax needs no transposes and
all matmul operands are bf16 so the PE streams at 1 row/cycle at any clock):
  xT   = dma-transpose(x)        (hardware xbar transpose, bf16)
  qkT  = xT + peT                (pos-enc, host-precomputed constant, DVE)
  qT   = Wq.T @ x_pe             via matmul(lhsT=Wq, rhs=qkT)
  kT   = Wk.T @ x_pe             "
  v    = x @ Wv                  via matmul(lhsT=xT, rhs=Wv)
  per head h:
    sT[m,l] = sum_d kT[d,m] qT[d,l]      matmul(lhsT=kT_h, rhs=qT_h)
    eT      = exp(0.125 * sT)            ScalarE, PSUM->SBUF, bf16 out
    O[l,d]+Z = eT.T @ [v_h | 1]          matmul(lhsT=eT, rhs=v_aug)
    out_h   = O * (1/Z)                  DVE reciprocal + tensor_scalar
Bias handling: bq added to qT during PSUM drain (per-partition scalar, in
fp32 before the bf16 rounding); bk only shifts each score row by a constant
(softmax-invariant) so it is dropped; bv shifts the output by exactly
repeat(bv, 64) because softmax rows sum to 1, added on the host.
"""

import numpy as np
import ml_dtypes

import concourse.bass as bass
import concourse.mybir as mybir
import concourse.tile as tile
from concourse import bacc
from concourse.bass_utils import run_bass_kernel_spmd

B, L, D = 8, 1024, 256
H, HD = 8, 64
J = H * HD  # 512
TEMPERATURE = 10000.0

f32 = mybir.dt.float32
bf16 = mybir.dt.bfloat16

_CACHE = {}
LAST_RESULT = None  # BassKernelResults of the most recent run (for profiling)
TRACE = False


def _emit(tc, aps):
    nc = tc.nc
    Exp = mybir.ActivationFunctionType.Exp
    x, wq, wk, wv, bqc, pet, out = (
        aps["x"], aps["wq"], aps["wk"], aps["wv"], aps["bqc"], aps["pet"], aps["out"],
    )

    # DRAM views: partition-major tilings
    petr = pet.rearrange("(t p) l -> t p l", p=128)      # [2, 128, 1024]
    wqr = wq.rearrange("(t p) j -> t p j", p=128)        # [2, 128, 512]
    wkr = wk.rearrange("(t p) j -> t p j", p=128)
    wvr = wv.rearrange("(t p) j -> t p j", p=128)
    outr = out.rearrange("(n p) j -> p n j", p=128)      # [128, 8, 512]

    import contextlib
    ctx = contextlib.ExitStack()
    persist = ctx.enter_context(tc.tile_pool(name="persist", bufs=1))
    epool = ctx.enter_context(tc.tile_pool(name="epool", bufs=24))
    rpool = ctx.enter_context(tc.tile_pool(name="rpool", bufs=4))
    mm_ps = ctx.enter_context(tc.tile_pool(name="mmps", bufs=2, space="PSUM"))
    s_ps = ctx.enter_context(tc.tile_pool(name="sps", bufs=3, space="PSUM"))

    # --- inputs to SBUF ---
    # x transposed via the hardware xbar DMA transpose (bf16-only path).
    xT = [persist.tile([128, 1024], bf16, name=f"xT{t}") for t in range(2)]
    for t in range(2):
        nc.sync.dma_start_transpose(xT[t][:], x[:, t * 128:(t + 1) * 128])

    w_sb = {}
    for wname in ("wq", "wk", "wv"):
        w_sb[wname] = [
            persist.tile([128, 512], bf16, name=f"{wname}_sb{t}") for t in range(2)
        ]
    for t in range(2):
        nc.sync.dma_start(out=w_sb["wq"][t][:], in_=wqr[t])

    pe_sb = [persist.tile([128, 1024], bf16, name=f"pe_sb{t}") for t in range(2)]
    for t in range(2):
        for q in range(2):
            sl = slice(q * 512, (q + 1) * 512)
            nc.gpsimd.dma_start(out=pe_sb[t][:, sl], in_=petr[t, :, sl])

    bq_sb = persist.tile([128, 4], f32, name="bq_sb")
    nc.gpsimd.dma_start(out=bq_sb[:], in_=bqc[:, :])

    for wname, wr in (("wk", wkr), ("wv", wvr)):
        for t in range(2):
            nc.gpsimd.dma_start(out=w_sb[wname][t][:], in_=wr[t])

    # --- qkT = xT + peT ---
    qkT = [persist.tile([128, 1024], bf16, name=f"qkT{t}") for t in range(2)]
    for t in range(2):
        nc.vector.tensor_add(qkT[t][:], xT[t][:], pe_sb[t][:])

    # --- QKV projections ---
    qT = [persist.tile([128, 1024], bf16, name=f"qT{j}") for j in range(4)]
    kT = [persist.tile([128, 1024], bf16, name=f"kT{j}") for j in range(4)]
    v_sb = [persist.tile([128, 8, 65], bf16, name=f"v_sb{m}") for m in range(8)]

    def qk_proj(j, which):
        dst, wname = (qT, "wq") if which == "q" else (kT, "wk")
        for l2 in range(2):
            pq = mm_ps.tile([128, 512], f32, tag="mm", name="pq")
            for c2 in range(2):
                nc.tensor.matmul(
                    pq[:],
                    lhsT=w_sb[wname][c2][:, j * 128:(j + 1) * 128],
                    rhs=qkT[c2][:, l2 * 512:(l2 + 1) * 512],
                    start=(c2 == 0),
                    stop=(c2 == 1),
                )
            dsl = slice(l2 * 512, (l2 + 1) * 512)
            if which == "q":
                nc.vector.tensor_scalar_add(dst[j][:, dsl], pq[:], bq_sb[:, j:j + 1])
            else:
                nc.vector.tensor_copy(dst[j][:, dsl], pq[:])

    def v_proj(m):
        pv = mm_ps.tile([128, 512], f32, tag="mm", name="pv")
        for c2 in range(2):
            nc.tensor.matmul(
                pv[:],
                lhsT=xT[c2][:, m * 128:(m + 1) * 128],
                rhs=w_sb["wv"][c2][:],
                start=(c2 == 0),
                stop=(c2 == 1),
            )
        nc.vector.tensor_copy(
            v_sb[m][:, :, 0:64], pv.rearrange("p (h d) -> p h d", h=8)
        )
        nc.vector.memset(v_sb[m][:, :, 64:65], 1.0)

    # --- attention, software-pipelined so the in-order PE stream never
    # blocks on exp: S(h+1) matmuls are emitted before O(h) matmuls ---
    out_sb = persist.tile([128, 8, 512], f32, name="out_sb")
    es_by_head = {}

    def emit_S(h):
        j2, po = h // 2, 64 * (h % 2)
        es = []
        for mc in range(8):
            ps = s_ps.tile([128, 1024], f32, tag="s", name="ps")
            for l2 in range(2):
                nc.tensor.matmul(
                    ps[:, l2 * 512:(l2 + 1) * 512],
                    lhsT=kT[j2][po:po + 64, mc * 128:(mc + 1) * 128],
                    rhs=qT[j2][po:po + 64, l2 * 512:(l2 + 1) * 512],
                    start=True,
                    stop=True,
                )
            e = epool.tile([128, 1024], bf16, tag="e", name="e")
            nc.scalar.activation(e[:], ps[:], Exp, scale=float(HD) ** -0.5)
            es.append(e)
        es_by_head[h] = es

    def emit_O(h):
        es = es_by_head.pop(h)
        for lc in range(8):
            pO = mm_ps.tile([128, 65], f32, tag="mm", name="pO")
            for mc in range(8):
                nc.tensor.matmul(
                    pO[:],
                    lhsT=es[mc][:, lc * 128:(lc + 1) * 128],
                    rhs=v_sb[mc][:, h, :],
                    start=(mc == 0),
                    stop=(mc == 7),
                )
            rc = rpool.tile([128, 1], f32, tag="rc", name="rc")
            nc.vector.reciprocal(rc[:], pO[:, 64:65])
            nc.vector.tensor_scalar_mul(
                out_sb[:, lc, h * 64:(h + 1) * 64], pO[:, 0:64], rc[:]
            )
        hsl = slice(h * 64, (h + 1) * 64)
        for g in range(4):
            eng = nc.sync if g % 2 == 0 else nc.gpsimd
            eng.dma_start(
                out=outr[:, 2 * g:2 * g + 2, hsl],
                in_=out_sb[:, 2 * g:2 * g + 2, hsl],
            )

    qk_proj(0, "q")
    qk_proj(0, "k")
    for m in range(8):
        v_proj(m)
    emit_S(0)
    qk_proj(1, "q")
    qk_proj(1, "k")
    emit_S(1)
    emit_O(0)
    qk_proj(2, "q")
    qk_proj(2, "k")
    emit_S(2)
    emit_O(1)
    qk_proj(3, "q")
    qk_proj(3, "k")
    for h in range(3, 8):
        emit_S(h)
        emit_O(h - 1)
    emit_O(7)
    ctx.close()


def _build():
    if "nc" in _CACHE:
        return _CACHE["nc"]
    nc = bacc.Bacc("TRN2", target_bir_lowering=False, debug=False, num_devices=8)
    aps = {
        "x": nc.dram_tensor("x", [L, D], bf16, kind="ExternalInput").ap(),
        "wq": nc.dram_tensor("wq", [D, J], bf16, kind="ExternalInput").ap(),
        "wk": nc.dram_tensor("wk", [D, J], bf16, kind="ExternalInput").ap(),
        "wv": nc.dram_tensor("wv", [D, J], bf16, kind="ExternalInput").ap(),
        "bqc": nc.dram_tensor("bqc", [128, 4], f32, kind="ExternalInput").ap(),
        "pet": nc.dram_tensor("pet", [D, L], bf16, kind="ExternalInput").ap(),
        "out": nc.dram_tensor("out", [L, J], f32, kind="ExternalOutput").ap(),
    }
    with tile.TileContext(nc) as tc:
        _emit(tc, aps)
    nc.compile()
    _CACHE["nc"] = nc
    return nc


def _pe_T():
    embed = np.arange(L, dtype=np.float32)
    dim_t = np.arange(D, dtype=np.float32)
    dim_t = (np.float32(TEMPERATURE) ** (2.0 * np.floor(dim_t / 2.0) / np.float32(D))).astype(np.float32)
    pos = embed[:, None] / dim_t  # [L, D]
    pe = np.stack([np.sin(pos[:, 0::2]), np.cos(pos[:, 1::2])], axis=2).reshape(L, D)
    return np.ascontiguousarray(pe.T.astype(np.float32))  # [D, L]


def kernel(**inputs):
    global LAST_RESULT
    bf = ml_dtypes.bfloat16
    x = np.asarray(inputs["x"], dtype=np.float32).astype(bf)
    wq = np.ascontiguousarray(np.asarray(inputs["Wq"], dtype=np.float32).astype(bf))
    wk = np.ascontiguousarray(np.asarray(inputs["Wk"], dtype=np.float32).astype(bf))
    wv = np.ascontiguousarray(np.asarray(inputs["Wv"], dtype=np.float32).astype(bf))
    bq = np.asarray(inputs["bq"], dtype=np.float32)
    bv = np.asarray(inputs["bv"], dtype=np.float32)

    nc = _build()
    bqc = np.ascontiguousarray(np.repeat(bq, HD).reshape(4, 128).T)  # [128, 4]
    pet = _pe_T().astype(bf)
    base = {"wq": wq, "wk": wk, "wv": wv, "bqc": bqc, "pet": pet}
    in_maps = [{**base, "x": np.ascontiguousarray(x[b])} for b in range(B)]
    res = run_bass_kernel_spmd(
        nc, in_maps, core_ids=list(range(B)), trace=TRACE
    )
    LAST_RESULT = res
    out = np.stack([res.results[b]["out"] for b in range(B)]).astype(np.float32)
    out += np.repeat(bv, HD)[None, None, :]
    return out


# revision 8
# speedup vs baseline: 1.5941x; 1.1059x over previous
"""Multi-head distance (attention) layer on 8 TRN2 NeuronCores.

Sharding: data-parallel over batch. B=8 -> one batch element per core.
Each core computes a full multi-head self-attention for its [L=1024, D=256]
slice with H=8 heads of dim 64. No collectives needed.

Per-core algorithm (all layouts chosen so softmax needs no transposes and
all matmul operands are bf16 so the PE streams at 1 row/cycle at any clock):
  xT   = dma-transpose(x)        (hardware xbar transpose, bf16)
  qkT  = xT + peT                (pos-enc, host-precomputed constant, DVE)
  qT   = Wq.T @ x_pe             via matmul(lhsT=Wq, rhs=qkT)
  kT   = Wk.T @ x_pe             "
  v    = x @ Wv                  via matmul(lhsT=xT, rhs=Wv)
  per head h:
    sT[m,l] = sum_d kT[d,m] qT[d,l]      matmul(lhsT=kT_h, rhs=qT_h)
    eT      = exp(0.125 * sT)            ScalarE, PSUM->SBUF, bf16 out
    O[l,d]+Z = eT.T @ [v_h | 1]          matmul(lhsT=eT, rhs=v_aug)
    out_h   = O * (1/Z)                  DVE reciprocal + tensor_scalar
Bias handling: bq added to qT during PSUM drain (per-partition scalar, in
fp32 before the bf16 rounding); bk only shifts each score row by a constant
(softmax-invariant) so it is dropped; bv shifts the output by exactly
repeat(bv, 64) because softmax rows sum to 1, added on the host.
"""

import numpy as np
import ml_dtypes

import concourse.bass as bass
import concourse.mybir as mybir
import concourse.tile as tile
from concourse import bacc
from concourse.bass_utils import run_bass_kernel_spmd

B, L, D = 8, 1024, 256
H, HD = 8, 64
J = H * HD  # 512
TEMPERATURE = 10000.0

f32 = mybir.dt.float32
bf16 = mybir.dt.bfloat16

_CACHE = {}
LAST_RESULT = None  # BassKernelResults of the most recent run (for profiling)
TRACE = False


def _emit(tc, aps):
    nc = tc.nc
    Exp = mybir.ActivationFunctionType.Exp
    x, wq, wk, wv, bqc, pet, out = (
        aps["x"], aps["wq"], aps["wk"], aps["wv"], aps["bqc"], aps["pet"], aps["out"],
    )

    # DRAM views: partition-major tilings
    petr = pet.rearrange("(t p) l -> t p l", p=128)      # [2, 128, 1024]
    wqr = wq.rearrange("(t p) j -> t p j", p=128)        # [2, 128, 512]
    wkr = wk.rearrange("(t p) j -> t p j", p=128)
    wvr = wv.rearrange("(t p) j -> t p j", p=128)
    outr = out.rearrange("(n p) j -> p n j", p=128)      # [128, 8, 512]

    import contextlib
    ctx = contextlib.ExitStack()
    persist = ctx.enter_context(tc.tile_pool(name="persist", bufs=1))
    epool = ctx.enter_context(tc.tile_pool(name="epool", bufs=24))
    rpool = ctx.enter_context(tc.tile_pool(name="rpool", bufs=4))
    mm_ps = ctx.enter_context(tc.tile_pool(name="mmps", bufs=2, space="PSUM"))
    s_ps = ctx.enter_context(tc.tile_pool(name="sps", bufs=3, space="PSUM"))

    # --- inputs to SBUF ---
    # x transposed via the hardware xbar DMA transpose (bf16-only path).
    xT = [persist.tile([128, 1024], bf16, name=f"xT{t}") for t in range(2)]
    for t in range(2):
        nc.sync.dma_start_transpose(xT[t][:], x[:, t * 128:(t + 1) * 128])

    w_sb = {}
    for wname in ("wq", "wk", "wv"):
        w_sb[wname] = [
            persist.tile([128, 512], bf16, name=f"{wname}_sb{t}") for t in range(2)
        ]
    for t in range(2):
        nc.sync.dma_start(out=w_sb["wq"][t][:], in_=wqr[t])

    pe_sb = [persist.tile([128, 1024], bf16, name=f"pe_sb{t}") for t in range(2)]
    for t in range(2):
        for q in range(2):
            sl = slice(q * 512, (q + 1) * 512)
            nc.gpsimd.dma_start(out=pe_sb[t][:, sl], in_=petr[t, :, sl])

    bq_sb = persist.tile([128, 4], f32, name="bq_sb")
    nc.gpsimd.dma_start(out=bq_sb[:], in_=bqc[:, :])

    for wname, wr in (("wk", wkr), ("wv", wvr)):
        for t in range(2):
            nc.gpsimd.dma_start(out=w_sb[wname][t][:], in_=wr[t])

    # --- qkT = xT + peT ---
    qkT = [persist.tile([128, 1024], bf16, name=f"qkT{t}") for t in range(2)]
    for t in range(2):
        nc.vector.tensor_add(qkT[t][:], xT[t][:], pe_sb[t][:])

    # --- QKV projections ---
    qT = [persist.tile([128, 1024], bf16, name=f"qT{j}") for j in range(4)]
    # Per-head zero-padded K tiles: head h occupies its 64 rows, the other
    # head's rows are zero, so the S matmul can contract over K=128 (K=64
    # matmuls stream at half rate on the PE).
    kTz = [persist.tile([128, 1024], bf16, name=f"kTz{h}") for h in range(8)]
    for h in range(8):
        nc.vector.memset(kTz[h][:], 0.0)
    v_sb = [persist.tile([128, 8, 65], bf16, name=f"v_sb{m}") for m in range(8)]

    def qk_proj(j, which):
        wname = "wq" if which == "q" else "wk"
        for l2 in range(2):
            pq = mm_ps.tile([128, 512], f32, tag="mm", name="pq")
            for c2 in range(2):
                nc.tensor.matmul(
                    pq[:],
                    lhsT=w_sb[wname][c2][:, j * 128:(j + 1) * 128],
                    rhs=qkT[c2][:, l2 * 512:(l2 + 1) * 512],
                    start=(c2 == 0),
                    stop=(c2 == 1),
                )
            dsl = slice(l2 * 512, (l2 + 1) * 512)
            if which == "q":
                nc.vector.tensor_scalar_add(qT[j][:, dsl], pq[:], bq_sb[:, j:j + 1])
            else:
                nc.vector.tensor_copy(kTz[2 * j][0:64, dsl], pq[0:64, :])
                nc.vector.tensor_copy(kTz[2 * j + 1][64:128, dsl], pq[64:128, :])

    def v_proj(m):
        pv = mm_ps.tile([128, 512], f32, tag="mm", name="pv")
        for c2 in range(2):
            nc.tensor.matmul(
                pv[:],
                lhsT=xT[c2][:, m * 128:(m + 1) * 128],
                rhs=w_sb["wv"][c2][:],
                start=(c2 == 0),
                stop=(c2 == 1),
            )
        nc.vector.tensor_copy(
            v_sb[m][:, :, 0:64], pv.rearrange("p (h d) -> p h d", h=8)
        )
        nc.vector.memset(v_sb[m][:, :, 64:65], 1.0)

    # --- attention, software-pipelined so the in-order PE stream never
    # blocks on exp: S(h+1) matmuls are emitted before O(h) matmuls ---
    out_sb = persist.tile([128, 8, 512], f32, name="out_sb")
    es_by_head = {}

    def emit_S(h):
        j2 = h // 2
        es = []
        for mc in range(8):
            ps = s_ps.tile([128, 1024], f32, tag="s", name="ps")
            for l2 in range(2):
                nc.tensor.matmul(
                    ps[:, l2 * 512:(l2 + 1) * 512],
                    lhsT=kTz[h][:, mc * 128:(mc + 1) * 128],
                    rhs=qT[j2][:, l2 * 512:(l2 + 1) * 512],
                    start=True,
                    stop=True,
                )
            e = epool.tile([128, 1024], bf16, tag="e", name="e")
            nc.scalar.activation(e[:], ps[:], Exp, scale=float(HD) ** -0.5)
            es.append(e)
        es_by_head[h] = es

    def emit_O(h):
        es = es_by_head.pop(h)
        for lc in range(8):
            pO = mm_ps.tile([128, 65], f32, tag="mm", name="pO")
            for mc in range(8):
                nc.tensor.matmul(
                    pO[:],
                    lhsT=es[mc][:, lc * 128:(lc + 1) * 128],
                    rhs=v_sb[mc][:, h, :],
                    start=(mc == 0),
                    stop=(mc == 7),
                )
            rc = rpool.tile([128, 1], f32, tag="rc", name="rc")
            nc.vector.reciprocal(rc[:], pO[:, 64:65])
            nc.vector.tensor_scalar_mul(
                out_sb[:, lc, h * 64:(h + 1) * 64], pO[:, 0:64], rc[:]
            )
        hsl = slice(h * 64, (h + 1) * 64)
        for g in range(4):
            eng = nc.sync if g % 2 == 0 else nc.gpsimd
            eng.dma_start(
                out=outr[:, 2 * g:2 * g + 2, hsl],
                in_=out_sb[:, 2 * g:2 * g + 2, hsl],
            )

    qk_proj(0, "q")
    qk_proj(0, "k")
    for m in range(8):
        v_proj(m)
    emit_S(0)
    qk_proj(1, "q")
    qk_proj(1, "k")
    emit_S(1)
    emit_O(0)
    qk_proj(2, "q")
    qk_proj(2, "k")
    emit_S(2)
    emit_O(1)
    qk_proj(3, "q")
    qk_proj(3, "k")
    for h in range(3, 8):
        emit_S(h)
        emit_O(h - 1)
    emit_O(7)
    ctx.close()


def _build():
    if "nc" in _CACHE:
        return _CACHE["nc"]
    nc = bacc.Bacc("TRN2", target_bir_lowering=False, debug=False, num_devices=8)
    aps = {
        "x": nc.dram_tensor("x", [L, D], bf16, kind="ExternalInput").ap(),
        "wq": nc.dram_tensor("wq", [D, J], bf16, kind="ExternalInput").ap(),
        "wk": nc.dram_tensor("wk", [D, J], bf16, kind="ExternalInput").ap(),
        "wv": nc.dram_tensor("wv", [D, J], bf16, kind="ExternalInput").ap(),
        "bqc": nc.dram_tensor("bqc", [128, 4], f32, kind="ExternalInput").ap(),
        "pet": nc.dram_tensor("pet", [D, L], bf16, kind="ExternalInput").ap(),
        "out": nc.dram_tensor("out", [L, J], f32, kind="ExternalOutput").ap(),
    }
    with tile.TileContext(nc) as tc:
        _emit(tc, aps)
    nc.compile()
    _CACHE["nc"] = nc
    return nc


def _pe_T():
    embed = np.arange(L, dtype=np.float32)
    dim_t = np.arange(D, dtype=np.float32)
    dim_t = (np.float32(TEMPERATURE) ** (2.0 * np.floor(dim_t / 2.0) / np.float32(D))).astype(np.float32)
    pos = embed[:, None] / dim_t  # [L, D]
    pe = np.stack([np.sin(pos[:, 0::2]), np.cos(pos[:, 1::2])], axis=2).reshape(L, D)
    return np.ascontiguousarray(pe.T.astype(np.float32))  # [D, L]


def kernel(**inputs):
    global LAST_RESULT
    bf = ml_dtypes.bfloat16
    x = np.asarray(inputs["x"], dtype=np.float32).astype(bf)
    wq = np.ascontiguousarray(np.asarray(inputs["Wq"], dtype=np.float32).astype(bf))
    wk = np.ascontiguousarray(np.asarray(inputs["Wk"], dtype=np.float32).astype(bf))
    wv = np.ascontiguousarray(np.asarray(inputs["Wv"], dtype=np.float32).astype(bf))
    bq = np.asarray(inputs["bq"], dtype=np.float32)
    bv = np.asarray(inputs["bv"], dtype=np.float32)

    nc = _build()
    bqc = np.ascontiguousarray(np.repeat(bq, HD).reshape(4, 128).T)  # [128, 4]
    pet = _pe_T().astype(bf)
    base = {"wq": wq, "wk": wk, "wv": wv, "bqc": bqc, "pet": pet}
    in_maps = [{**base, "x": np.ascontiguousarray(x[b])} for b in range(B)]
    res = run_bass_kernel_spmd(
        nc, in_maps, core_ids=list(range(B)), trace=TRACE
    )
    LAST_RESULT = res
    out = np.stack([res.results[b]["out"] for b in range(B)]).astype(np.float32)
    out += np.repeat(bv, HD)[None, None, :]
    return out
